# revision 5
# baseline (speedup 1.0000x reference)
"""DISCO S2 conv (DiscreteContinuousConvS2) Trainium2 Bass kernel, v2.

Algorithm (spectral-longitude DISCO, validated vs reference):
  psi applied with 360 longitude shifts == circular correlation along lon;
  psi is even in lon so its lon-DFT is real.  Per core:
    A. einsum over C_in:  xw[po, la, m] = x[:, la, po].T @ w2   (m = k*48+oh)
       - x-slice is the matmul stationary so xw lands po-major (no transpose)
    B. forward rDFT over lon as matmul:  xh[f, la, m] (f = 362 stacked re/im)
    D. per-(k,dla) diagonal spectral multiply-accumulate; only 10 of the 14
       (k,dla) pairs are nonzero (boundary rings are exactly 0).  fp16 on
       DVE (2x mode) + GpSimd, two partial accumulators.
    E. inverse rDFT as matmul, accumulating both partials in PSUM; output
       rows are flat (ho, oh) per latitude-third.
  Sharding: 8 cores = (batch b in 0..3) x (C_out half), fully data-parallel,
  no collectives.  Latitude processed in three ho-thirds with +-3 la halo.
"""
import sys
import numpy as np

for _p in ("/opt/trn_rl_repo",):
    if _p not in sys.path:
        sys.path.insert(0, _p)

NLAT, NLON, NF, FDIM = 181, 360, 181, 362
K, B, CIN, COUT, OH = 2, 4, 96, 96, 48
M = OH * K  # 96 channels after einsum, layout m = k*48 + oh
# (ho0, ho1, la0, la1): output-lat third and its +-3-halo input-lat range
THIRDS = [(0, 61, 0, 64), (61, 121, 58, 124), (121, 181, 118, 181)]
HOW = 61  # max ho rows per third (phat DRAM padding)
PS = [(0, 128), (128, 256), (256, 360)]   # po chunks (contraction for B)
FS = [(0, 128), (128, 256), (256, 362)]   # f chunks
# 10 nonzero (k, dla) pairs; (0,+-2) and (1,+-4) are exactly zero.
# First pair of each engine list must cover the full ho-window (dla=0).
DVE_PAIRS = [(0, 0), (0, -1), (0, 1), (1, -1), (1, 1), (1, -2), (1, 2), (1, 3)]
POOL_PAIRS = [(1, 0), (1, -3)]
NZ = DVE_PAIRS + POOL_PAIRS
NPAIR = len(NZ)
LA_G = 5      # A-stage la group (PSUM batching)
BLK_LA = 5    # B-stage moving block = BLK_LA*96 = 480 rows <= 512 psum bank

_CACHE = {}


def _host_prep(weight, psi_vals, k_idx, ho_idx, lat_in, lon_in):
    dla_all = lat_in.astype(np.int64) - ho_idx.astype(np.int64)
    P = np.zeros((K, 9, NLAT, NLON), dtype=np.float64)
    np.add.at(P, (k_idx, dla_all + 4, ho_idx, lon_in), psi_vals.astype(np.float64))
    f = np.arange(NF)
    ang = 2 * np.pi * np.outer(np.arange(NLON), f) / NLON          # [360,181]
    dfwd = np.concatenate([np.cos(ang), -np.sin(ang)], axis=1)     # [360,362]
    cf = np.full(NF, 2.0 / NLON)
    cf[0] = 1.0 / NLON
    cf[NF - 1] = 1.0 / NLON
    dinv = np.concatenate([cf[:, None] * np.cos(ang.T),
                           -cf[:, None] * np.sin(ang.T)], axis=0)  # [362,360]
    dinv[NF, :] = 0.0
    dinv[2 * NF - 1, :] = 0.0
    phat_all = P @ np.cos(ang)                                     # [K,9,ho,181]
    # per-third phat, fp16, duplicated x2 along a trailing dim so the DVE
    # broadcast AP keeps an innermost +-1 16-bit run (2x perf mode)
    phat2 = np.zeros((3, FDIM, NPAIR, HOW, 2), dtype=np.float16)
    for ti, (ho0, ho1, _, _) in enumerate(THIRDS):
        w = ho1 - ho0
        for ip, (k, dla) in enumerate(NZ):
            pT = phat_all[k, dla + 4, ho0:ho1, :].T                # [181f, w]
            phat2[ti, :NF, ip, :w, 0] = pT
            phat2[ti, NF:, ip, :w, 0] = pT
    phat2[..., 1] = phat2[..., 0]
    return (np.ascontiguousarray(dfwd.astype(np.float16)),
            np.ascontiguousarray(dinv.astype(np.float16)),
            np.ascontiguousarray(phat2))


def _sub_ap(base, elem_off, dims):
    """Free-dim rewrite of an AP: keep partition dim, set free dims/offset."""
    import concourse.bass as bass
    return bass.AP(tensor=base.tensor, offset=base.offset + elem_off,
                   ap=[list(base.ap[0])] + [list(d) for d in dims])


def _build_nc():
    import concourse.bass as bass
    import concourse.bacc as bacc
    import concourse.tile as tile
    from concourse import mybir

    f32 = mybir.dt.float32
    f16 = mybir.dt.float16

    nc = bacc.Bacc("TRN2", target_bir_lowering=False, debug=False)

    x_in = nc.dram_tensor("x_in", [CIN, NLAT, NLON], f16, kind="ExternalInput").ap()
    w2_in = nc.dram_tensor("w2_in", [CIN, M], f16, kind="ExternalInput").ap()
    dfwd_in = nc.dram_tensor("dfwd_in", [NLON, FDIM], f16, kind="ExternalInput").ap()
    dinv_in = nc.dram_tensor("dinv_in", [FDIM, NLON], f16, kind="ExternalInput").ap()
    phat_in = nc.dram_tensor("phat_in", [3, FDIM, NPAIR * HOW * 2], f16,
                             kind="ExternalInput").ap()
    out_d = nc.dram_tensor("out", [OH * NLAT, NLON], f32, kind="ExternalOutput").ap()

    from contextlib import ExitStack
    with tile.TileContext(nc) as tc, ExitStack() as es:
        consts = es.enter_context(tc.tile_pool(name="consts", bufs=1))
        x_pool = es.enter_context(tc.tile_pool(name="x", bufs=2))
        xwT_pool = es.enter_context(tc.tile_pool(name="xwT", bufs=1))
        xh_pool = es.enter_context(tc.tile_pool(name="xh", bufs=2))
        yhd_pool = es.enter_context(tc.tile_pool(name="yhd", bufs=2))
        yhp_pool = es.enter_context(tc.tile_pool(name="yhp", bufs=1))
        tmpd_pool = es.enter_context(tc.tile_pool(name="tmpd", bufs=1))
        tmpp_pool = es.enter_context(tc.tile_pool(name="tmpp", bufs=1))
        phat_pool = es.enter_context(tc.tile_pool(name="phat", bufs=2))
        est_pool = es.enter_context(tc.tile_pool(name="est", bufs=2))
        ps_a = es.enter_context(tc.tile_pool(name="ps_a", bufs=3, space=bass.MemorySpace.PSUM))
        ps_be = es.enter_context(tc.tile_pool(name="ps_be", bufs=2, space=bass.MemorySpace.PSUM))

        w2_sb = consts.tile([CIN, M], f16)
        nc.sync.dma_start(out=w2_sb[:, :], in_=w2_in[:, :])
        dfwd_sb = consts.tile([128, 3, FDIM], f16)
        for j, (p0, p1) in enumerate(PS):
            nc.sync.dma_start(out=dfwd_sb[:p1 - p0, j, :], in_=dfwd_in[p0:p1, :])
        dinv_sb = consts.tile([128, 3, NLON], f16)
        for t, (f0, f1) in enumerate(FS):
            nc.sync.dma_start(out=dinv_sb[:f1 - f0, t, :], in_=dinv_in[f0:f1, :])

        row_base = 0
        for ti, (ho0, ho1, la0, la1) in enumerate(THIRDS):
            la_w = la1 - la0
            how = ho1 - ho0
            flat_y = how * OH

            phat_sb = phat_pool.tile([128, 3, NPAIR, HOW, 2], f16, tag="phat")
            for t, (f0, f1) in enumerate(FS):
                nc.sync.dma_start(out=phat_sb[:f1 - f0, t, :, :, :],
                                  in_=phat_in[ti, f0:f1, :])

            # ---- stage A: einsum, out xwT[po, j, la, m] fp16 ----
            xwT = xwT_pool.tile([128, 3, la_w, M], f16, tag="xwT")
            for g in range(la0, la1, LA_G):
                gn = min(LA_G, la1 - g)
                xg = x_pool.tile([CIN, LA_G, NLON], f16, tag="xg")
                nc.sync.dma_start(out=xg[:, :gn, :], in_=x_in[:, g:g + gn, :])
                for j, (p0, p1) in enumerate(PS):
                    psa = ps_a.tile([128, 512], f32, tag="ps_a")
                    for il in range(gn):
                        nc.tensor.matmul(
                            psa[:p1 - p0, il * M:il * M + M],
                            xg[:, il, p0:p1],
                            w2_sb[:, :],
                            start=True, stop=True)
                    nc.scalar.copy(
                        xwT[:p1 - p0, j, g - la0:g - la0 + gn, :],
                        _sub_ap(psa[:p1 - p0, 0:1], 0, [[M, gn], [1, M]]))

            # ---- stage B: forward DFT, xh[f, t, la*96+m] fp16 ----
            xh = xh_pool.tile([128, 3, la_w * M], f16, tag="xh")
            for t, (f0, f1) in enumerate(FS):
                fsz = f1 - f0
                for l0 in range(0, la_w, BLK_LA):
                    ln = min(BLK_LA, la_w - l0)
                    n = ln * M
                    psb = ps_be.tile([128, 512], f32, tag="ps_b")
                    for j, (p0, p1) in enumerate(PS):
                        nc.tensor.matmul(
                            psb[:fsz, :n],
                            dfwd_sb[:p1 - p0, j, f0:f1],
                            xwT[:p1 - p0, j, l0:l0 + ln, :],
                            start=(j == 0), stop=(j == 2))
                    nc.scalar.copy(xh[:fsz, t, l0 * M:l0 * M + n], psb[:fsz, :n])

            # ---- stage D: spectral multiply-accumulate (DVE + Pool) ----
            yh_d = yhd_pool.tile([128, 3, flat_y], f16, tag="yh_d")
            yh_p = yhp_pool.tile([128, 3, flat_y], f16, tag="yh_p")
            tmp_d = tmpd_pool.tile([128, HOW * OH], f16, tag="tmp_d")
            tmp_p = tmpp_pool.tile([128, HOW * OH], f16, tag="tmp_p")

            for eng, pairs, yh, tmp in (
                    (nc.vector, DVE_PAIRS, yh_d, tmp_d),
                    (nc.gpsimd, POOL_PAIRS, yh_p, tmp_p)):
                for ipl, (k, dla) in enumerate(pairs):
                    ip = NZ.index((k, dla))
                    ho_lo = max(ho0, -dla)
                    ho_hi = min(ho1, NLAT - dla)
                    w = ho_hi - ho_lo
                    a = ho_lo + dla - la0
                    hl = ho_lo - ho0
                    assert w > 0 and a >= 0 and a + w <= la_w
                    dims_o = [[OH, w], [2, 24], [1, 2]]
                    for t in range(3):
                        fsz = FS[t][1] - FS[t][0]
                        in0 = _sub_ap(xh[:fsz, t, 0:1], a * M + k * OH,
                                      [[M, w], [2, 24], [1, 2]])
                        pb = phat_sb[:fsz, t, ip, hl:hl + w, :]
                        in1 = bass.AP(tensor=pb.tensor, offset=pb.offset,
                                      ap=[list(pb.ap[0]), list(pb.ap[1]),
                                          [0, 24], list(pb.ap[2])])
                        if ipl == 0:
                            outp = _sub_ap(yh[:fsz, t, 0:1], hl * OH, dims_o)
                            eng.tensor_mul(outp, in0, in1)
                        else:
                            tm = _sub_ap(tmp[:fsz, 0:1], 0, dims_o)
                            eng.tensor_mul(tm, in0, in1)
                            yslc = _sub_ap(yh[:fsz, t, 0:1], hl * OH, dims_o)
                            eng.tensor_add(yslc, yslc, tm)

            # ---- stage E: inverse DFT + merge partials + store ----
            for c0 in range(0, flat_y, 128):
                cn = min(128, flat_y - c0)
                pse = ps_be.tile([128, NLON], f32, tag="ps_e")
                idx = 0
                for yh in (yh_d, yh_p):
                    for t, (f0, f1) in enumerate(FS):
                        fsz = f1 - f0
                        nc.tensor.matmul(
                            pse[:cn, :],
                            yh[:fsz, t, c0:c0 + cn],
                            dinv_sb[:fsz, t, :],
                            start=(idx == 0), stop=(idx == 5))
                        idx += 1
                o_sb = est_pool.tile([128, NLON], f32, tag="o_sb")
                nc.scalar.copy(o_sb[:cn, :], pse[:cn, :])
                r0 = row_base + c0
                nc.sync.dma_start(out=out_d[r0:r0 + cn, :], in_=o_sb[:cn, :])
            row_base += flat_y

    nc.compile()
    return nc


def _get_runner(n_cores=8):
    """Build (once) a jitted shard_map runner for the compiled Bass module."""
    if "runner" in _CACHE:
        return _CACHE["runner"]
    import jax
    import jax.numpy as jnp
    from jax.sharding import Mesh, PartitionSpec, NamedSharding
    from jax.experimental.shard_map import shard_map
    from concourse import bass2jax, mybir

    if "nc" not in _CACHE:
        _CACHE["nc"] = _build_nc()
    nc = _CACHE["nc"]
    bass2jax.install_neuronx_cc_hook()

    partition_name = (nc.partition_id_tensor.name
                      if nc.partition_id_tensor else None)
    in_names, out_names, out_avals = [], [], []
    for alloc in nc.m.functions[0].allocations:
        if not isinstance(alloc, mybir.MemoryLocationSet):
            continue
        name = alloc.memorylocations[0].name
        if alloc.kind == "ExternalInput":
            if name != partition_name:
                in_names.append(name)
        elif alloc.kind == "ExternalOutput":
            out_names.append(name)
            out_avals.append(jax.core.ShapedArray(
                tuple(alloc.tensor_shape), mybir.dt.np(alloc.dtype)))
    n_params = len(in_names)
    n_outs = len(out_avals)
    all_names = in_names + out_names
    if partition_name is not None:
        all_names = all_names + [partition_name]

    def _body(*args):
        operands = list(args)
        if partition_name is not None:
            operands.append(bass2jax.partition_id_tensor())
        outs = bass2jax._bass_exec_p.bind(
            *operands,
            out_avals=tuple(out_avals),
            in_names=tuple(all_names),
            out_names=tuple(out_names),
            lowering_input_output_aliases=(),
            sim_require_finite=True,
            sim_require_nnan=True,
            nc=nc,
        )
        return tuple(outs)

    devices = jax.devices()[:n_cores]
    mesh = Mesh(np.asarray(devices), ("core",))
    spec = PartitionSpec("core")
    sharding = NamedSharding(mesh, spec)
    donate = tuple(range(n_params, n_params + n_outs))
    sharded = jax.jit(
        shard_map(_body, mesh=mesh, in_specs=(spec,) * (n_params + n_outs),
                  out_specs=(spec,) * n_outs, check_rep=False),
        donate_argnums=donate, keep_unused=True)
    zero_shapes = [(n_cores * a.shape[0], *a.shape[1:]) for a in out_avals]
    zero_dtypes = [a.dtype for a in out_avals]
    make_zeros = jax.jit(
        lambda: tuple(jnp.zeros(s, d) for s, d in zip(zero_shapes, zero_dtypes)),
        out_shardings=(sharding,) * n_outs)
    runner = {
        "sharded": sharded, "make_zeros": make_zeros, "sharding": sharding,
        "in_names": in_names, "out_names": out_names, "out_avals": out_avals,
        "n_cores": n_cores,
    }
    _CACHE["runner"] = runner
    return runner


def _device_inputs(x, weight, psi_arrays):
    """Concatenated-global per-parameter arrays, device_put with sharding."""
    import jax
    dfwd, dinv, phat2 = _host_prep(weight, *psi_arrays)
    phat_flat = phat2.reshape(3, FDIM, NPAIR * HOW * 2)
    x16 = x.astype(np.float16)
    per_core = {"x_in": [], "w2_in": [], "dfwd_in": [], "dinv_in": [], "phat_in": []}
    for s in range(8):
        b, ohf = s // 2, s % 2
        o_sl = slice(OH * ohf, OH * ohf + OH)
        # m = k*48 + oh  (k-major)
        w2 = np.ascontiguousarray(
            weight[o_sl].transpose(1, 2, 0).reshape(CIN, M).astype(np.float16))
        per_core["x_in"].append(x16[b])
        per_core["w2_in"].append(w2)
        per_core["dfwd_in"].append(dfwd)
        per_core["dinv_in"].append(dinv)
        per_core["phat_in"].append(phat_flat)
    runner = _get_runner()
    concat = {k: np.concatenate(v, axis=0) for k, v in per_core.items()}
    return [jax.device_put(concat[name], runner["sharding"])
            for name in runner["in_names"]]


def _run_device(dev_in):
    runner = _get_runner()
    zeros = runner["make_zeros"]()
    return runner["sharded"](*dev_in, *zeros)


def kernel(x, weight, bias, psi_vals, k_idx, ho_idx, lat_in_idx, lon_in_idx):
    x = np.ascontiguousarray(np.asarray(x, dtype=np.float32))
    weight = np.asarray(weight, dtype=np.float32)
    bias = np.asarray(bias, dtype=np.float32)
    psi_arrays = (np.asarray(psi_vals), np.asarray(k_idx), np.asarray(ho_idx),
                  np.asarray(lat_in_idx), np.asarray(lon_in_idx))

    dev_in = _device_inputs(x, weight, psi_arrays)
    out_arrs = _run_device(dev_in)
    runner = _get_runner()
    a0 = runner["out_avals"][0]
    res0 = np.asarray(out_arrs[0]).reshape(8, *a0.shape)

    out = np.empty((B, COUT, NLAT, NLON), dtype=np.float32)
    for s in range(8):
        b, ohf = s // 2, s % 2
        r = res0[s]
        parts = []
        base = 0
        for (ho0, ho1, _, _) in THIRDS:
            how = ho1 - ho0
            blk = r[base:base + how * OH].reshape(how, OH, NLON)
            parts.append(blk.transpose(1, 0, 2))
            base += how * OH
        out[b, OH * ohf:OH * ohf + OH] = np.concatenate(parts, axis=1)
    if np.any(bias):
        out += bias[None, :, None, None]
    return out


# revision 19
# speedup vs baseline: 4.7765x; 4.7765x over previous
"""DISCO S2 conv (DiscreteContinuousConvS2) Trainium2 Bass kernel, v2.

Algorithm (spectral-longitude DISCO, validated vs reference):
  psi applied with 360 longitude shifts == circular correlation along lon;
  psi is even in lon so its lon-DFT is real.  Per core:
    A. einsum over C_in:  xw[po, la, m] = x[:, la, po].T @ w2   (m = k*48+oh)
       - x-slice is the matmul stationary so xw lands po-major (no transpose)
    B. forward rDFT over lon as matmul:  xh[f, la, m] (f = 362 stacked re/im)
    D. per-(k,dla) diagonal spectral multiply-accumulate; only 10 of the 14
       (k,dla) pairs are nonzero (boundary rings are exactly 0).  fp16 on
       DVE (2x mode) + GpSimd, two partial accumulators.
    E. inverse rDFT as matmul, accumulating both partials in PSUM; output
       rows are flat (ho, oh) per latitude-third.
  Sharding: 8 cores = (batch b in 0..3) x (C_out half), fully data-parallel,
  no collectives.  Latitude processed in three ho-thirds with +-3 la halo.
"""
import sys
import numpy as np

for _p in ("/opt/trn_rl_repo",):
    if _p not in sys.path:
        sys.path.insert(0, _p)

NLAT, NLON, NF, FDIM = 181, 360, 181, 362
K, B, CIN, COUT, OH = 2, 4, 96, 96, 48
M = OH * K  # 96 channels after einsum, layout m = k*48 + oh
# (ho0, ho1, la0, la1): output-lat segment and its +-3-halo input-lat range.
# First segment is small so the A/B lead-in before DVE work starts is short.
THIRDS = [(0, 16, 0, 19), (16, 58, 13, 61), (58, 99, 55, 102),
          (99, 140, 96, 143), (140, 181, 137, 181)]
NSEG = len(THIRDS)
HOW = 42  # max ho rows per segment (phat DRAM padding)
PS = [(0, 128), (128, 256), (256, 360)]   # po chunks (contraction for B)
FS = [(0, 128), (128, 256), (256, 362)]   # f chunks
# 10 nonzero (k, dla) pairs; (0,+-2) and (1,+-4) are exactly zero.
# First pair must cover the full ho-window (dla=0).  All on DVE: concurrent
# GpSimd elementwise work contends on SBUF and quarters DVE throughput.
NZ = [(0, 0), (1, 0), (0, -1), (0, 1), (1, -1), (1, 1),
      (1, -2), (1, 2), (1, -3), (1, 3)]
NPAIR = len(NZ)
LA_G = 5      # A-stage la group (PSUM batching)
BLK_LA = 5    # B-stage moving block = 480 rows <= 512 psum bank

_CACHE = {}


def _host_prep(weight, psi_vals, k_idx, ho_idx, lat_in, lon_in):
    dla_all = lat_in.astype(np.int64) - ho_idx.astype(np.int64)
    P = np.zeros((K, 9, NLAT, NLON), dtype=np.float64)
    np.add.at(P, (k_idx, dla_all + 4, ho_idx, lon_in), psi_vals.astype(np.float64))
    f = np.arange(NF)
    ang = 2 * np.pi * np.outer(np.arange(NLON), f) / NLON          # [360,181]
    dfwd = np.concatenate([np.cos(ang), -np.sin(ang)], axis=1)     # [360,362]
    cf = np.full(NF, 2.0 / NLON)
    cf[0] = 1.0 / NLON
    cf[NF - 1] = 1.0 / NLON
    dinv = np.concatenate([cf[:, None] * np.cos(ang.T),
                           -cf[:, None] * np.sin(ang.T)], axis=0)  # [362,360]
    dinv[NF, :] = 0.0
    dinv[2 * NF - 1, :] = 0.0
    phat_all = P @ np.cos(ang)                                     # [K,9,ho,181]
    # per-third phat, fp16, duplicated x4 along a trailing dim so the DVE
    # broadcast AP keeps a longer innermost 16-bit run (2x perf mode, fewer
    # inner-dim restarts)
    phat2 = np.zeros((NSEG, FDIM, NPAIR, HOW, 4), dtype=np.float16)
    for ti, (ho0, ho1, _, _) in enumerate(THIRDS):
        w = ho1 - ho0
        for ip, (k, dla) in enumerate(NZ):
            pT = phat_all[k, dla + 4, ho0:ho1, :].T                # [181f, w]
            phat2[ti, :NF, ip, :w, 0] = pT
            phat2[ti, NF:, ip, :w, 0] = pT
    for r in range(1, 4):
        phat2[..., r] = phat2[..., 0]
    return (np.ascontiguousarray(dfwd.astype(np.float16)),
            np.ascontiguousarray(dinv.astype(np.float16)),
            np.ascontiguousarray(phat2))


def _sub_ap(base, elem_off, dims):
    """Free-dim rewrite of an AP: keep partition dim, set free dims/offset."""
    import concourse.bass as bass
    return bass.AP(tensor=base.tensor, offset=base.offset + elem_off,
                   ap=[list(base.ap[0])] + [list(d) for d in dims])


def _build_nc():
    import concourse.bass as bass
    import concourse.bacc as bacc
    import concourse.tile as tile
    from concourse import mybir

    f32 = mybir.dt.float32
    f16 = mybir.dt.float16

    nc = bacc.Bacc("TRN2", target_bir_lowering=False, debug=False)

    x_in = nc.dram_tensor("x_in", [CIN, NLAT, NLON], f16, kind="ExternalInput").ap()
    w2_in = nc.dram_tensor("w2_in", [CIN, M], f16, kind="ExternalInput").ap()
    dfwd_in = nc.dram_tensor("dfwd_in", [NLON, FDIM], f16, kind="ExternalInput").ap()
    dinv_in = nc.dram_tensor("dinv_in", [FDIM, NLON], f16, kind="ExternalInput").ap()
    phat_in = nc.dram_tensor("phat_in", [NSEG, FDIM, NPAIR * HOW * 4], f16,
                             kind="ExternalInput").ap()
    out_d = nc.dram_tensor("out", [OH * NLAT, NLON], f32, kind="ExternalOutput").ap()

    from contextlib import ExitStack
    with tile.TileContext(nc) as tc, ExitStack() as es:
        consts = es.enter_context(tc.tile_pool(name="consts", bufs=1))
        x_pool = es.enter_context(tc.tile_pool(name="x", bufs=2))
        xwT_pool = es.enter_context(tc.tile_pool(name="xwT", bufs=1))
        xh_pool = es.enter_context(tc.tile_pool(name="xh", bufs=2))
        yhd_pool = es.enter_context(tc.tile_pool(name="yhd", bufs=3))
        tmpd_pool = es.enter_context(tc.tile_pool(name="tmpd", bufs=1))
        phat_pool = es.enter_context(tc.tile_pool(name="phat", bufs=2))
        est_pool = es.enter_context(tc.tile_pool(name="est", bufs=4))
        ps_a = es.enter_context(tc.tile_pool(name="ps_a", bufs=3, space=bass.MemorySpace.PSUM))
        ps_b = es.enter_context(tc.tile_pool(name="ps_b", bufs=3, space=bass.MemorySpace.PSUM))
        ps_e = es.enter_context(tc.tile_pool(name="ps_e", bufs=2, space=bass.MemorySpace.PSUM))

        w2_sb = consts.tile([CIN, M], f16)
        nc.sync.dma_start(out=w2_sb[:, :], in_=w2_in[:, :])
        dfwd_sb = consts.tile([128, 3, FDIM], f16)
        for j, (p0, p1) in enumerate(PS):
            nc.sync.dma_start(out=dfwd_sb[:p1 - p0, j, :], in_=dfwd_in[p0:p1, :])
        dinv_sb = consts.tile([128, 3, NLON], f16)
        for t, (f0, f1) in enumerate(FS):
            nc.sync.dma_start(out=dinv_sb[:f1 - f0, t, :], in_=dinv_in[f0:f1, :])

        def emit_e(yh, flat_y, row0):
            """Inverse DFT + store for one completed third."""
            for c0 in range(0, flat_y, 128):
                cn = min(128, flat_y - c0)
                pse = ps_e.tile([128, NLON], f32, tag="ps_e")
                for t, (f0, f1) in enumerate(FS):
                    nc.tensor.matmul(
                        pse[:cn, :],
                        yh[:f1 - f0, t, c0:c0 + cn],
                        dinv_sb[:f1 - f0, t, :],
                        start=(t == 0), stop=(t == 2))
                o_sb = est_pool.tile([128, NLON], f32, tag="o_sb")
                nc.scalar.copy(o_sb[:cn, :], pse[:cn, :])
                nc.sync.dma_start(out=out_d[row0 + c0:row0 + c0 + cn, :],
                                  in_=o_sb[:cn, :])

        phat0 = phat_pool.tile([128, 3, NPAIR, HOW, 4], f16, tag="phat")
        for t, (f0, f1) in enumerate(FS):
            nc.sync.dma_start(out=phat0[:f1 - f0, t, :, :, :],
                              in_=phat_in[0, f0:f1, :])

        pending_e = []
        row_base = 0
        for ti, (ho0, ho1, la0, la1) in enumerate(THIRDS):
            la_w = la1 - la0
            how = ho1 - ho0
            flat_y = how * OH

            # ---- stage A: einsum, out xwT[po, j, la, m] fp16 ----
            xwT = xwT_pool.tile([128, 3, la_w, M], f16, tag="xwT")
            for g in range(la0, la1, LA_G):
                gn = min(LA_G, la1 - g)
                xg = x_pool.tile([CIN, LA_G, NLON], f16, tag="xg")
                nc.sync.dma_start(out=xg[:, :gn, :], in_=x_in[:, g:g + gn, :])
                for j, (p0, p1) in enumerate(PS):
                    psa = ps_a.tile([128, 512], f32, tag="ps_a")
                    for il in range(gn):
                        nc.tensor.matmul(
                            psa[:p1 - p0, il * M:il * M + M],
                            xg[:, il, p0:p1],
                            w2_sb[:, :],
                            start=True, stop=True)
                    src_ap = _sub_ap(psa[:p1 - p0, 0:1], 0, [[M, gn], [1, M]])
                    dst_ap = xwT[:p1 - p0, j, g - la0:g - la0 + gn, :]
                    if ti == 0:
                        nc.vector.tensor_copy(dst_ap, src_ap)
                    else:
                        nc.scalar.copy(dst_ap, src_ap)

            # ---- stage B: forward DFT, xh[f, t, la*96+m] fp16 ----
            xh = xh_pool.tile([128, 3, la_w * M], f16, tag="xh")
            for t, (f0, f1) in enumerate(FS):
                fsz = f1 - f0
                for l0 in range(0, la_w, BLK_LA):
                    ln = min(BLK_LA, la_w - l0)
                    n = ln * M
                    psb = ps_b.tile([128, 512], f32, tag="ps_b")
                    for j, (p0, p1) in enumerate(PS):
                        nc.tensor.matmul(
                            psb[:fsz, :n],
                            dfwd_sb[:p1 - p0, j, f0:f1],
                            xwT[:p1 - p0, j, l0:l0 + ln, :],
                            start=(j == 0), stop=(j == 2))
                    if ti == 0 and t == 0:
                        nc.vector.tensor_copy(xh[:fsz, t, l0 * M:l0 * M + n],
                                              psb[:fsz, :n])
                    else:
                        nc.scalar.copy(xh[:fsz, t, l0 * M:l0 * M + n],
                                       psb[:fsz, :n])

            # phat DMA after the A/B x-feed so it never delays it on the queue
            # (segment 0's was loaded up-front)
            if ti == 0:
                phat_sb = phat0
            else:
                phat_sb = phat_pool.tile([128, 3, NPAIR, HOW, 4], f16, tag="phat")
                for t, (f0, f1) in enumerate(FS):
                    nc.sync.dma_start(out=phat_sb[:f1 - f0, t, :, :, :],
                                      in_=phat_in[ti, f0:f1, :])

            # ---- stage E, two segments behind: keeps the PE queue (and the
            # scalar-engine evac FIFO) from stalling ahead of A/B work ----
            if len(pending_e) >= 2:
                emit_e(*pending_e.pop(0))

            # ---- stage D: spectral multiply-accumulate (DVE only) ----
            yh_d = yhd_pool.tile([128, 3, flat_y], f16, tag="yh_d")
            tmp_d = tmpd_pool.tile([128, HOW * OH], f16, tag="tmp_d")
            for t in range(3):
                fsz = FS[t][1] - FS[t][0]
                for ip, (k, dla) in enumerate(NZ):
                    ho_lo = max(ho0, -dla)
                    ho_hi = min(ho1, NLAT - dla)
                    w = ho_hi - ho_lo
                    a = ho_lo + dla - la0
                    hl = ho_lo - ho0
                    assert w > 0 and a >= 0 and a + w <= la_w
                    dims_o = [[OH, w], [4, 12], [1, 4]]
                    in0 = _sub_ap(xh[:fsz, t, 0:1], a * M + k * OH,
                                  [[M, w], [4, 12], [1, 4]])
                    pb = phat_sb[:fsz, t, ip, hl:hl + w, :]
                    in1 = bass.AP(tensor=pb.tensor, offset=pb.offset,
                                  ap=[list(pb.ap[0]), list(pb.ap[1]),
                                      [0, 12], list(pb.ap[2])])
                    if ip == 0:
                        outp = _sub_ap(yh_d[:fsz, t, 0:1], hl * OH, dims_o)
                        nc.vector.tensor_mul(outp, in0, in1)
                    else:
                        tm = _sub_ap(tmp_d[:fsz, 0:1], 0, dims_o)
                        nc.vector.tensor_mul(tm, in0, in1)
                        yflat = _sub_ap(yh_d[:fsz, t, 0:1], hl * OH,
                                        [[1, w * OH]])
                        tflat = _sub_ap(tmp_d[:fsz, 0:1], 0, [[1, w * OH]])
                        nc.vector.tensor_add(yflat, yflat, tflat)

            pending_e.append((yh_d, flat_y, row_base))
            row_base += flat_y
        for pe_args in pending_e:
            emit_e(*pe_args)

    nc.compile()
    return nc


def _get_runner(n_cores=8):
    """Build (once) a jitted shard_map runner for the compiled Bass module."""
    if "runner" in _CACHE:
        return _CACHE["runner"]
    import jax
    import jax.numpy as jnp
    from jax.sharding import Mesh, PartitionSpec, NamedSharding
    from jax.experimental.shard_map import shard_map
    from concourse import bass2jax, mybir

    if "nc" not in _CACHE:
        _CACHE["nc"] = _build_nc()
    nc = _CACHE["nc"]
    bass2jax.install_neuronx_cc_hook()

    partition_name = (nc.partition_id_tensor.name
                      if nc.partition_id_tensor else None)
    in_names, out_names, out_avals = [], [], []
    for alloc in nc.m.functions[0].allocations:
        if not isinstance(alloc, mybir.MemoryLocationSet):
            continue
        name = alloc.memorylocations[0].name
        if alloc.kind == "ExternalInput":
            if name != partition_name:
                in_names.append(name)
        elif alloc.kind == "ExternalOutput":
            out_names.append(name)
            out_avals.append(jax.core.ShapedArray(
                tuple(alloc.tensor_shape), mybir.dt.np(alloc.dtype)))
    n_params = len(in_names)
    n_outs = len(out_avals)
    all_names = in_names + out_names
    if partition_name is not None:
        all_names = all_names + [partition_name]

    def _body(*args):
        operands = list(args)
        if partition_name is not None:
            operands.append(bass2jax.partition_id_tensor())
        outs = bass2jax._bass_exec_p.bind(
            *operands,
            out_avals=tuple(out_avals),
            in_names=tuple(all_names),
            out_names=tuple(out_names),
            lowering_input_output_aliases=(),
            sim_require_finite=True,
            sim_require_nnan=True,
            nc=nc,
        )
        return tuple(outs)

    devices = jax.devices()[:n_cores]
    mesh = Mesh(np.asarray(devices), ("core",))
    spec = PartitionSpec("core")
    sharding = NamedSharding(mesh, spec)
    donate = tuple(range(n_params, n_params + n_outs))
    sharded = jax.jit(
        shard_map(_body, mesh=mesh, in_specs=(spec,) * (n_params + n_outs),
                  out_specs=(spec,) * n_outs, check_rep=False),
        donate_argnums=donate, keep_unused=True)
    zero_shapes = [(n_cores * a.shape[0], *a.shape[1:]) for a in out_avals]
    zero_dtypes = [a.dtype for a in out_avals]
    make_zeros = jax.jit(
        lambda: tuple(jnp.zeros(s, d) for s, d in zip(zero_shapes, zero_dtypes)),
        out_shardings=(sharding,) * n_outs)
    runner = {
        "sharded": sharded, "make_zeros": make_zeros, "sharding": sharding,
        "in_names": in_names, "out_names": out_names, "out_avals": out_avals,
        "n_cores": n_cores,
    }
    _CACHE["runner"] = runner
    return runner


def _device_inputs(x, weight, psi_arrays):
    """Concatenated-global per-parameter arrays, device_put with sharding."""
    import jax
    dfwd, dinv, phat2 = _host_prep(weight, *psi_arrays)
    phat_flat = phat2.reshape(NSEG, FDIM, NPAIR * HOW * 4)
    x16 = x.astype(np.float16)
    per_core = {"x_in": [], "w2_in": [], "dfwd_in": [], "dinv_in": [], "phat_in": []}
    for s in range(8):
        b, ohf = s // 2, s % 2
        o_sl = slice(OH * ohf, OH * ohf + OH)
        # m = k*48 + oh  (k-major)
        w2 = np.ascontiguousarray(
            weight[o_sl].transpose(1, 2, 0).reshape(CIN, M).astype(np.float16))
        per_core["x_in"].append(x16[b])
        per_core["w2_in"].append(w2)
        per_core["dfwd_in"].append(dfwd)
        per_core["dinv_in"].append(dinv)
        per_core["phat_in"].append(phat_flat)
    runner = _get_runner()
    concat = {k: np.concatenate(v, axis=0) for k, v in per_core.items()}
    return [jax.device_put(concat[name], runner["sharding"])
            for name in runner["in_names"]]


def _run_device(dev_in):
    runner = _get_runner()
    zeros = runner["make_zeros"]()
    return runner["sharded"](*dev_in, *zeros)


def kernel(x, weight, bias, psi_vals, k_idx, ho_idx, lat_in_idx, lon_in_idx):
    x = np.ascontiguousarray(np.asarray(x, dtype=np.float32))
    weight = np.asarray(weight, dtype=np.float32)
    bias = np.asarray(bias, dtype=np.float32)
    psi_arrays = (np.asarray(psi_vals), np.asarray(k_idx), np.asarray(ho_idx),
                  np.asarray(lat_in_idx), np.asarray(lon_in_idx))

    dev_in = _device_inputs(x, weight, psi_arrays)
    out_arrs = _run_device(dev_in)
    runner = _get_runner()
    a0 = runner["out_avals"][0]
    res0 = np.asarray(out_arrs[0]).reshape(8, *a0.shape)

    out = np.empty((B, COUT, NLAT, NLON), dtype=np.float32)
    for s in range(8):
        b, ohf = s // 2, s % 2
        r = res0[s]
        parts = []
        base = 0
        for (ho0, ho1, _, _) in THIRDS:
            how = ho1 - ho0
            blk = r[base:base + how * OH].reshape(how, OH, NLON)
            parts.append(blk.transpose(1, 0, 2))
            base += how * OH
        out[b, OH * ohf:OH * ohf + OH] = np.concatenate(parts, axis=1)
    if np.any(bias):
        out += bias[None, :, None, None]
    return out


# revision 20
# speedup vs baseline: 4.7951x; 1.0039x over previous
"""DISCO S2 conv (DiscreteContinuousConvS2) Trainium2 Bass kernel, v2.

Algorithm (spectral-longitude DISCO, validated vs reference):
  psi applied with 360 longitude shifts == circular correlation along lon;
  psi is even in lon so its lon-DFT is real.  Per core:
    A. einsum over C_in:  xw[po, la, m] = x[:, la, po].T @ w2   (m = k*48+oh)
       - x-slice is the matmul stationary so xw lands po-major (no transpose)
    B. forward rDFT over lon as matmul:  xh[f, la, m] (f = 362 stacked re/im)
    D. per-(k,dla) diagonal spectral multiply-accumulate; only 10 of the 14
       (k,dla) pairs are nonzero (boundary rings are exactly 0).  fp16 on
       DVE (2x mode) + GpSimd, two partial accumulators.
    E. inverse rDFT as matmul, accumulating both partials in PSUM; output
       rows are flat (ho, oh) per latitude-third.
  Sharding: 8 cores = (batch b in 0..3) x (C_out half), fully data-parallel,
  no collectives.  Latitude processed in three ho-thirds with +-3 la halo.
"""
import sys
import numpy as np

for _p in ("/opt/trn_rl_repo",):
    if _p not in sys.path:
        sys.path.insert(0, _p)

NLAT, NLON, NF, FDIM = 181, 360, 181, 362
K, B, CIN, COUT, OH = 2, 4, 96, 96, 48
M = OH * K  # 96 channels after einsum, layout m = k*48 + oh
# (ho0, ho1, la0, la1): output-lat segment and its +-3-halo input-lat range.
# First segment is small so the A/B lead-in before DVE work starts is short.
THIRDS = [(0, 16, 0, 19), (16, 61, 13, 64), (61, 106, 58, 109),
          (106, 151, 103, 154), (151, 181, 148, 181)]
NSEG = len(THIRDS)
HOW = 45  # max ho rows per segment (phat DRAM padding)
PS = [(0, 128), (128, 256), (256, 360)]   # po chunks (contraction for B)
FS = [(0, 128), (128, 256), (256, 362)]   # f chunks
# 10 nonzero (k, dla) pairs; (0,+-2) and (1,+-4) are exactly zero.
# First pair must cover the full ho-window (dla=0).  All on DVE: concurrent
# GpSimd elementwise work contends on SBUF and quarters DVE throughput.
NZ = [(0, 0), (1, 0), (0, -1), (0, 1), (1, -1), (1, 1),
      (1, -2), (1, 2), (1, -3), (1, 3)]
NPAIR = len(NZ)
LA_G = 5      # A-stage la group (PSUM batching)
BLK_LA = 5    # B-stage moving block = 480 rows <= 512 psum bank

_CACHE = {}


def _host_prep(weight, psi_vals, k_idx, ho_idx, lat_in, lon_in):
    dla_all = lat_in.astype(np.int64) - ho_idx.astype(np.int64)
    P = np.zeros((K, 9, NLAT, NLON), dtype=np.float64)
    np.add.at(P, (k_idx, dla_all + 4, ho_idx, lon_in), psi_vals.astype(np.float64))
    f = np.arange(NF)
    ang = 2 * np.pi * np.outer(np.arange(NLON), f) / NLON          # [360,181]
    dfwd = np.concatenate([np.cos(ang), -np.sin(ang)], axis=1)     # [360,362]
    cf = np.full(NF, 2.0 / NLON)
    cf[0] = 1.0 / NLON
    cf[NF - 1] = 1.0 / NLON
    dinv = np.concatenate([cf[:, None] * np.cos(ang.T),
                           -cf[:, None] * np.sin(ang.T)], axis=0)  # [362,360]
    dinv[NF, :] = 0.0
    dinv[2 * NF - 1, :] = 0.0
    phat_all = P @ np.cos(ang)                                     # [K,9,ho,181]
    # per-third phat, fp16, duplicated x4 along a trailing dim so the DVE
    # broadcast AP keeps a longer innermost 16-bit run (2x perf mode, fewer
    # inner-dim restarts)
    phat2 = np.zeros((NSEG, FDIM, NPAIR, HOW, 4), dtype=np.float16)
    for ti, (ho0, ho1, _, _) in enumerate(THIRDS):
        w = ho1 - ho0
        for ip, (k, dla) in enumerate(NZ):
            pT = phat_all[k, dla + 4, ho0:ho1, :].T                # [181f, w]
            phat2[ti, :NF, ip, :w, 0] = pT
            phat2[ti, NF:, ip, :w, 0] = pT
    for r in range(1, 4):
        phat2[..., r] = phat2[..., 0]
    return (np.ascontiguousarray(dfwd.astype(np.float16)),
            np.ascontiguousarray(dinv.astype(np.float16)),
            np.ascontiguousarray(phat2))


def _sub_ap(base, elem_off, dims):
    """Free-dim rewrite of an AP: keep partition dim, set free dims/offset."""
    import concourse.bass as bass
    return bass.AP(tensor=base.tensor, offset=base.offset + elem_off,
                   ap=[list(base.ap[0])] + [list(d) for d in dims])


def _build_nc():
    import concourse.bass as bass
    import concourse.bacc as bacc
    import concourse.tile as tile
    from concourse import mybir

    f32 = mybir.dt.float32
    f16 = mybir.dt.float16

    nc = bacc.Bacc("TRN2", target_bir_lowering=False, debug=False)

    x_in = nc.dram_tensor("x_in", [CIN, NLAT, NLON], f16, kind="ExternalInput").ap()
    w2_in = nc.dram_tensor("w2_in", [CIN, M], f16, kind="ExternalInput").ap()
    dfwd_in = nc.dram_tensor("dfwd_in", [NLON, FDIM], f16, kind="ExternalInput").ap()
    dinv_in = nc.dram_tensor("dinv_in", [FDIM, NLON], f16, kind="ExternalInput").ap()
    phat_in = nc.dram_tensor("phat_in", [NSEG, FDIM, NPAIR * HOW * 4], f16,
                             kind="ExternalInput").ap()
    out_d = nc.dram_tensor("out", [OH * NLAT, NLON], f32, kind="ExternalOutput").ap()

    from contextlib import ExitStack
    with tile.TileContext(nc) as tc, ExitStack() as es:
        consts = es.enter_context(tc.tile_pool(name="consts", bufs=1))
        x_pool = es.enter_context(tc.tile_pool(name="x", bufs=2))
        xwT_pool = es.enter_context(tc.tile_pool(name="xwT", bufs=1))
        xh_pool = es.enter_context(tc.tile_pool(name="xh", bufs=2))
        yhd_pool = es.enter_context(tc.tile_pool(name="yhd", bufs=3))
        tmpd_pool = es.enter_context(tc.tile_pool(name="tmpd", bufs=1))
        phat_pool = es.enter_context(tc.tile_pool(name="phat", bufs=2))
        est_pool = es.enter_context(tc.tile_pool(name="est", bufs=4))
        ps_a = es.enter_context(tc.tile_pool(name="ps_a", bufs=3, space=bass.MemorySpace.PSUM))
        ps_b = es.enter_context(tc.tile_pool(name="ps_b", bufs=3, space=bass.MemorySpace.PSUM))
        ps_e = es.enter_context(tc.tile_pool(name="ps_e", bufs=2, space=bass.MemorySpace.PSUM))

        w2_sb = consts.tile([CIN, M], f16)
        nc.sync.dma_start(out=w2_sb[:, :], in_=w2_in[:, :])
        dfwd_sb = consts.tile([128, 3, FDIM], f16)
        for j, (p0, p1) in enumerate(PS):
            nc.sync.dma_start(out=dfwd_sb[:p1 - p0, j, :], in_=dfwd_in[p0:p1, :])
        dinv_sb = consts.tile([128, 3, NLON], f16)
        for t, (f0, f1) in enumerate(FS):
            nc.sync.dma_start(out=dinv_sb[:f1 - f0, t, :], in_=dinv_in[f0:f1, :])

        def emit_e(yh, flat_y, row0):
            """Inverse DFT + store for one completed third."""
            for c0 in range(0, flat_y, 128):
                cn = min(128, flat_y - c0)
                pse = ps_e.tile([128, NLON], f32, tag="ps_e")
                for t, (f0, f1) in enumerate(FS):
                    nc.tensor.matmul(
                        pse[:cn, :],
                        yh[:f1 - f0, t, c0:c0 + cn],
                        dinv_sb[:f1 - f0, t, :],
                        start=(t == 0), stop=(t == 2))
                o_sb = est_pool.tile([128, NLON], f32, tag="o_sb")
                nc.scalar.copy(o_sb[:cn, :], pse[:cn, :])
                nc.sync.dma_start(out=out_d[row0 + c0:row0 + c0 + cn, :],
                                  in_=o_sb[:cn, :])

        phat0 = phat_pool.tile([128, 3, NPAIR, HOW, 4], f16, tag="phat")
        nc.sync.dma_start(out=phat0[:FS[0][1], 0, :, :, :],
                          in_=phat_in[0, FS[0][0]:FS[0][1], :])

        pending_e = []
        row_base = 0
        for ti, (ho0, ho1, la0, la1) in enumerate(THIRDS):
            la_w = la1 - la0
            how = ho1 - ho0
            flat_y = how * OH

            # ---- stage A: einsum, out xwT[po, j, la, m] fp16 ----
            xwT = xwT_pool.tile([128, 3, la_w, M], f16, tag="xwT")
            for g in range(la0, la1, LA_G):
                gn = min(LA_G, la1 - g)
                xg = x_pool.tile([CIN, LA_G, NLON], f16, tag="xg")
                nc.sync.dma_start(out=xg[:, :gn, :], in_=x_in[:, g:g + gn, :])
                for j, (p0, p1) in enumerate(PS):
                    psa = ps_a.tile([128, 512], f32, tag="ps_a")
                    for il in range(gn):
                        nc.tensor.matmul(
                            psa[:p1 - p0, il * M:il * M + M],
                            xg[:, il, p0:p1],
                            w2_sb[:, :],
                            start=True, stop=True)
                    src_ap = _sub_ap(psa[:p1 - p0, 0:1], 0, [[M, gn], [1, M]])
                    dst_ap = xwT[:p1 - p0, j, g - la0:g - la0 + gn, :]
                    if ti == 0:
                        nc.vector.tensor_copy(dst_ap, src_ap)
                    else:
                        nc.scalar.copy(dst_ap, src_ap)

            # ---- stage B: forward DFT, xh[f, t, la*96+m] fp16 ----
            xh = xh_pool.tile([128, 3, la_w * M], f16, tag="xh")
            for t, (f0, f1) in enumerate(FS):
                fsz = f1 - f0
                for l0 in range(0, la_w, BLK_LA):
                    ln = min(BLK_LA, la_w - l0)
                    n = ln * M
                    psb = ps_b.tile([128, 512], f32, tag="ps_b")
                    for j, (p0, p1) in enumerate(PS):
                        nc.tensor.matmul(
                            psb[:fsz, :n],
                            dfwd_sb[:p1 - p0, j, f0:f1],
                            xwT[:p1 - p0, j, l0:l0 + ln, :],
                            start=(j == 0), stop=(j == 2))
                    if ti == 0 and t == 0:
                        nc.vector.tensor_copy(xh[:fsz, t, l0 * M:l0 * M + n],
                                              psb[:fsz, :n])
                    else:
                        nc.scalar.copy(xh[:fsz, t, l0 * M:l0 * M + n],
                                       psb[:fsz, :n])

            # phat DMA after the A/B x-feed so it never delays it on the queue
            # (segment 0's was loaded up-front)
            if ti == 0:
                phat_sb = phat0
                for t, (f0, f1) in enumerate(FS):
                    if t > 0:
                        nc.sync.dma_start(out=phat0[:f1 - f0, t, :, :, :],
                                          in_=phat_in[0, f0:f1, :])
            else:
                phat_sb = phat_pool.tile([128, 3, NPAIR, HOW, 4], f16, tag="phat")
                for t, (f0, f1) in enumerate(FS):
                    nc.sync.dma_start(out=phat_sb[:f1 - f0, t, :, :, :],
                                      in_=phat_in[ti, f0:f1, :])

            # ---- stage E, two segments behind: keeps the PE queue (and the
            # scalar-engine evac FIFO) from stalling ahead of A/B work ----
            if len(pending_e) >= 2:
                emit_e(*pending_e.pop(0))

            # ---- stage D: spectral multiply-accumulate (DVE only) ----
            yh_d = yhd_pool.tile([128, 3, flat_y], f16, tag="yh_d")
            tmp_d = tmpd_pool.tile([128, HOW * OH], f16, tag="tmp_d")
            for t in range(3):
                fsz = FS[t][1] - FS[t][0]
                for ip, (k, dla) in enumerate(NZ):
                    ho_lo = max(ho0, -dla)
                    ho_hi = min(ho1, NLAT - dla)
                    w = ho_hi - ho_lo
                    a = ho_lo + dla - la0
                    hl = ho_lo - ho0
                    assert w > 0 and a >= 0 and a + w <= la_w
                    dims_o = [[OH, w], [4, 12], [1, 4]]
                    in0 = _sub_ap(xh[:fsz, t, 0:1], a * M + k * OH,
                                  [[M, w], [4, 12], [1, 4]])
                    pb = phat_sb[:fsz, t, ip, hl:hl + w, :]
                    in1 = bass.AP(tensor=pb.tensor, offset=pb.offset,
                                  ap=[list(pb.ap[0]), list(pb.ap[1]),
                                      [0, 12], list(pb.ap[2])])
                    if ip == 0:
                        outp = _sub_ap(yh_d[:fsz, t, 0:1], hl * OH, dims_o)
                        nc.vector.tensor_mul(outp, in0, in1)
                    else:
                        tm = _sub_ap(tmp_d[:fsz, 0:1], 0, dims_o)
                        nc.vector.tensor_mul(tm, in0, in1)
                        yflat = _sub_ap(yh_d[:fsz, t, 0:1], hl * OH,
                                        [[1, w * OH]])
                        tflat = _sub_ap(tmp_d[:fsz, 0:1], 0, [[1, w * OH]])
                        nc.vector.tensor_add(yflat, yflat, tflat)

            pending_e.append((yh_d, flat_y, row_base))
            row_base += flat_y
        for pe_args in pending_e:
            emit_e(*pe_args)

    nc.compile()
    return nc


def _get_runner(n_cores=8):
    """Build (once) a jitted shard_map runner for the compiled Bass module."""
    if "runner" in _CACHE:
        return _CACHE["runner"]
    import jax
    import jax.numpy as jnp
    from jax.sharding import Mesh, PartitionSpec, NamedSharding
    from jax.experimental.shard_map import shard_map
    from concourse import bass2jax, mybir

    if "nc" not in _CACHE:
        _CACHE["nc"] = _build_nc()
    nc = _CACHE["nc"]
    bass2jax.install_neuronx_cc_hook()

    partition_name = (nc.partition_id_tensor.name
                      if nc.partition_id_tensor else None)
    in_names, out_names, out_avals = [], [], []
    for alloc in nc.m.functions[0].allocations:
        if not isinstance(alloc, mybir.MemoryLocationSet):
            continue
        name = alloc.memorylocations[0].name
        if alloc.kind == "ExternalInput":
            if name != partition_name:
                in_names.append(name)
        elif alloc.kind == "ExternalOutput":
            out_names.append(name)
            out_avals.append(jax.core.ShapedArray(
                tuple(alloc.tensor_shape), mybir.dt.np(alloc.dtype)))
    n_params = len(in_names)
    n_outs = len(out_avals)
    all_names = in_names + out_names
    if partition_name is not None:
        all_names = all_names + [partition_name]

    def _body(*args):
        operands = list(args)
        if partition_name is not None:
            operands.append(bass2jax.partition_id_tensor())
        outs = bass2jax._bass_exec_p.bind(
            *operands,
            out_avals=tuple(out_avals),
            in_names=tuple(all_names),
            out_names=tuple(out_names),
            lowering_input_output_aliases=(),
            sim_require_finite=True,
            sim_require_nnan=True,
            nc=nc,
        )
        return tuple(outs)

    devices = jax.devices()[:n_cores]
    mesh = Mesh(np.asarray(devices), ("core",))
    spec = PartitionSpec("core")
    sharding = NamedSharding(mesh, spec)
    donate = tuple(range(n_params, n_params + n_outs))
    sharded = jax.jit(
        shard_map(_body, mesh=mesh, in_specs=(spec,) * (n_params + n_outs),
                  out_specs=(spec,) * n_outs, check_rep=False),
        donate_argnums=donate, keep_unused=True)
    zero_shapes = [(n_cores * a.shape[0], *a.shape[1:]) for a in out_avals]
    zero_dtypes = [a.dtype for a in out_avals]
    make_zeros = jax.jit(
        lambda: tuple(jnp.zeros(s, d) for s, d in zip(zero_shapes, zero_dtypes)),
        out_shardings=(sharding,) * n_outs)
    runner = {
        "sharded": sharded, "make_zeros": make_zeros, "sharding": sharding,
        "in_names": in_names, "out_names": out_names, "out_avals": out_avals,
        "n_cores": n_cores,
    }
    _CACHE["runner"] = runner
    return runner


def _device_inputs(x, weight, psi_arrays):
    """Concatenated-global per-parameter arrays, device_put with sharding."""
    import jax
    dfwd, dinv, phat2 = _host_prep(weight, *psi_arrays)
    phat_flat = phat2.reshape(NSEG, FDIM, NPAIR * HOW * 4)
    x16 = x.astype(np.float16)
    per_core = {"x_in": [], "w2_in": [], "dfwd_in": [], "dinv_in": [], "phat_in": []}
    for s in range(8):
        b, ohf = s // 2, s % 2
        o_sl = slice(OH * ohf, OH * ohf + OH)
        # m = k*48 + oh  (k-major)
        w2 = np.ascontiguousarray(
            weight[o_sl].transpose(1, 2, 0).reshape(CIN, M).astype(np.float16))
        per_core["x_in"].append(x16[b])
        per_core["w2_in"].append(w2)
        per_core["dfwd_in"].append(dfwd)
        per_core["dinv_in"].append(dinv)
        per_core["phat_in"].append(phat_flat)
    runner = _get_runner()
    concat = {k: np.concatenate(v, axis=0) for k, v in per_core.items()}
    return [jax.device_put(concat[name], runner["sharding"])
            for name in runner["in_names"]]


def _run_device(dev_in):
    runner = _get_runner()
    zeros = runner["make_zeros"]()
    return runner["sharded"](*dev_in, *zeros)


def kernel(x, weight, bias, psi_vals, k_idx, ho_idx, lat_in_idx, lon_in_idx):
    x = np.ascontiguousarray(np.asarray(x, dtype=np.float32))
    weight = np.asarray(weight, dtype=np.float32)
    bias = np.asarray(bias, dtype=np.float32)
    psi_arrays = (np.asarray(psi_vals), np.asarray(k_idx), np.asarray(ho_idx),
                  np.asarray(lat_in_idx), np.asarray(lon_in_idx))

    dev_in = _device_inputs(x, weight, psi_arrays)
    out_arrs = _run_device(dev_in)
    runner = _get_runner()
    a0 = runner["out_avals"][0]
    res0 = np.asarray(out_arrs[0]).reshape(8, *a0.shape)

    out = np.empty((B, COUT, NLAT, NLON), dtype=np.float32)
    for s in range(8):
        b, ohf = s // 2, s % 2
        r = res0[s]
        parts = []
        base = 0
        for (ho0, ho1, _, _) in THIRDS:
            how = ho1 - ho0
            blk = r[base:base + how * OH].reshape(how, OH, NLON)
            parts.append(blk.transpose(1, 0, 2))
            base += how * OH
        out[b, OH * ohf:OH * ohf + OH] = np.concatenate(parts, axis=1)
    if np.any(bias):
        out += bias[None, :, None, None]
    return out


# revision 21
# speedup vs baseline: 5.2889x; 1.1030x over previous
"""DISCO S2 conv (DiscreteContinuousConvS2) Trainium2 Bass kernel, v2.

Algorithm (spectral-longitude DISCO, validated vs reference):
  psi applied with 360 longitude shifts == circular correlation along lon;
  psi is even in lon so its lon-DFT is real.  Per core:
    A. einsum over C_in:  xw[po, la, m] = x[:, la, po].T @ w2   (m = k*48+oh)
       - x-slice is the matmul stationary so xw lands po-major (no transpose)
    B. forward rDFT over lon as matmul:  xh[f, la, m] (f = 362 stacked re/im)
    D. per-(k,dla) diagonal spectral multiply-accumulate; only 10 of the 14
       (k,dla) pairs are nonzero (boundary rings are exactly 0).  fp16 on
       DVE (2x mode) + GpSimd, two partial accumulators.
    E. inverse rDFT as matmul, accumulating both partials in PSUM; output
       rows are flat (ho, oh) per latitude-third.
  Sharding: 8 cores = (batch b in 0..3) x (C_out half), fully data-parallel,
  no collectives.  Latitude processed in three ho-thirds with +-3 la halo.
"""
import sys
import numpy as np

for _p in ("/opt/trn_rl_repo",):
    if _p not in sys.path:
        sys.path.insert(0, _p)

NLAT, NLON, NF, FDIM = 181, 360, 181, 362
K, B, CIN, COUT, OH = 2, 4, 96, 96, 48
M = OH * K  # 96 channels after einsum, layout m = k*48 + oh
# (ho0, ho1, la0, la1): output-lat segment and its +-3-halo input-lat range.
# First segment is small so the A/B lead-in before DVE work starts is short.
THIRDS = [(0, 16, 0, 19), (16, 61, 13, 64), (61, 106, 58, 109),
          (106, 151, 103, 154), (151, 181, 148, 181)]
NSEG = len(THIRDS)
HOW = 45  # max ho rows per segment (phat DRAM padding)
PS = [(0, 128), (128, 256), (256, 360)]   # po chunks (contraction for B)
FS = [(0, 128), (128, 256), (256, 362)]   # f chunks
# 10 nonzero (k, dla) pairs; (0,+-2) and (1,+-4) are exactly zero.
# First pair must cover the full ho-window (dla=0).  All on DVE: concurrent
# GpSimd elementwise work contends on SBUF and quarters DVE throughput.
NZ = [(0, 0), (1, 0), (0, -1), (0, 1), (1, -1), (1, 1),
      (1, -2), (1, 2), (1, -3), (1, 3)]
NPAIR = len(NZ)
LA_G = 5      # A-stage la group (PSUM batching)
BLK_LA = 5    # B-stage moving block = 480 rows <= 512 psum bank

_CACHE = {}


def _host_prep(weight, psi_vals, k_idx, ho_idx, lat_in, lon_in):
    dla_all = lat_in.astype(np.int64) - ho_idx.astype(np.int64)
    P = np.zeros((K, 9, NLAT, NLON), dtype=np.float64)
    np.add.at(P, (k_idx, dla_all + 4, ho_idx, lon_in), psi_vals.astype(np.float64))
    f = np.arange(NF)
    ang = 2 * np.pi * np.outer(np.arange(NLON), f) / NLON          # [360,181]
    dfwd = np.concatenate([np.cos(ang), -np.sin(ang)], axis=1)     # [360,362]
    cf = np.full(NF, 2.0 / NLON)
    cf[0] = 1.0 / NLON
    cf[NF - 1] = 1.0 / NLON
    dinv = np.concatenate([cf[:, None] * np.cos(ang.T),
                           -cf[:, None] * np.sin(ang.T)], axis=0)  # [362,360]
    dinv[NF, :] = 0.0
    dinv[2 * NF - 1, :] = 0.0
    phat_all = P @ np.cos(ang)                                     # [K,9,ho,181]
    # per-third phat, fp16, duplicated x4 along a trailing dim so the DVE
    # broadcast AP keeps a longer innermost 16-bit run (2x perf mode, fewer
    # inner-dim restarts)
    phat2 = np.zeros((NSEG, FDIM, NPAIR, HOW, 4), dtype=np.float16)
    for ti, (ho0, ho1, _, _) in enumerate(THIRDS):
        w = ho1 - ho0
        for ip, (k, dla) in enumerate(NZ):
            pT = phat_all[k, dla + 4, ho0:ho1, :].T                # [181f, w]
            phat2[ti, :NF, ip, :w, 0] = pT
            phat2[ti, NF:, ip, :w, 0] = pT
    for r in range(1, 4):
        phat2[..., r] = phat2[..., 0]
    return (np.ascontiguousarray(dfwd.astype(np.float16)),
            np.ascontiguousarray(dinv.astype(np.float16)),
            np.ascontiguousarray(phat2))


def _sub_ap(base, elem_off, dims):
    """Free-dim rewrite of an AP: keep partition dim, set free dims/offset."""
    import concourse.bass as bass
    return bass.AP(tensor=base.tensor, offset=base.offset + elem_off,
                   ap=[list(base.ap[0])] + [list(d) for d in dims])


def _build_nc():
    import concourse.bass as bass
    import concourse.bacc as bacc
    import concourse.tile as tile
    from concourse import mybir

    f32 = mybir.dt.float32
    f16 = mybir.dt.float16

    nc = bacc.Bacc("TRN2", target_bir_lowering=False, debug=False)

    x_in = nc.dram_tensor("x_in", [CIN, NLAT, NLON], f16, kind="ExternalInput").ap()
    w2_in = nc.dram_tensor("w2_in", [CIN, M], f16, kind="ExternalInput").ap()
    dfwd_in = nc.dram_tensor("dfwd_in", [NLON, FDIM], f16, kind="ExternalInput").ap()
    dinv_in = nc.dram_tensor("dinv_in", [FDIM, NLON], f16, kind="ExternalInput").ap()
    phat_in = nc.dram_tensor("phat_in", [NSEG, FDIM, NPAIR * HOW * 4], f16,
                             kind="ExternalInput").ap()
    out_d = nc.dram_tensor("out", [OH * NLAT, NLON], f32, kind="ExternalOutput").ap()

    from contextlib import ExitStack
    with tile.TileContext(nc) as tc, ExitStack() as es:
        consts = es.enter_context(tc.tile_pool(name="consts", bufs=1))
        x_pool = es.enter_context(tc.tile_pool(name="x", bufs=3))
        xwT_pool = es.enter_context(tc.tile_pool(name="xwT", bufs=1))
        xh_pool = es.enter_context(tc.tile_pool(name="xh", bufs=2))
        yhd_pool = es.enter_context(tc.tile_pool(name="yhd", bufs=3))
        tmpd_pool = es.enter_context(tc.tile_pool(name="tmpd", bufs=2))
        phat_pool = es.enter_context(tc.tile_pool(name="phat", bufs=2))
        est_pool = es.enter_context(tc.tile_pool(name="est", bufs=6))
        ps_a = es.enter_context(tc.tile_pool(name="ps_a", bufs=3, space=bass.MemorySpace.PSUM))
        ps_b = es.enter_context(tc.tile_pool(name="ps_b", bufs=3, space=bass.MemorySpace.PSUM))
        ps_e = es.enter_context(tc.tile_pool(name="ps_e", bufs=2, space=bass.MemorySpace.PSUM))

        w2_sb = consts.tile([CIN, M], f16)
        nc.sync.dma_start(out=w2_sb[:, :], in_=w2_in[:, :])
        dfwd_sb = consts.tile([128, 3, FDIM], f16)
        for j, (p0, p1) in enumerate(PS):
            nc.sync.dma_start(out=dfwd_sb[:p1 - p0, j, :], in_=dfwd_in[p0:p1, :])
        dinv_sb = consts.tile([128, 3, NLON], f16)
        for t, (f0, f1) in enumerate(FS):
            nc.sync.dma_start(out=dinv_sb[:f1 - f0, t, :], in_=dinv_in[f0:f1, :])

        def emit_e(yh, flat_y, row0):
            """Inverse DFT + store for one completed third."""
            for c0 in range(0, flat_y, 128):
                cn = min(128, flat_y - c0)
                pse = ps_e.tile([128, NLON], f32, tag="ps_e")
                for t, (f0, f1) in enumerate(FS):
                    nc.tensor.matmul(
                        pse[:cn, :],
                        yh[:f1 - f0, t, c0:c0 + cn],
                        dinv_sb[:f1 - f0, t, :],
                        start=(t == 0), stop=(t == 2))
                o_sb = est_pool.tile([128, NLON], f32, tag="o_sb")
                nc.scalar.copy(o_sb[:cn, :], pse[:cn, :])
                nc.sync.dma_start(out=out_d[row0 + c0:row0 + c0 + cn, :],
                                  in_=o_sb[:cn, :])

        phat0 = phat_pool.tile([128, 3, NPAIR, HOW, 4], f16, tag="phat")
        nc.sync.dma_start(out=phat0[:FS[0][1], 0, :, :, :],
                          in_=phat_in[0, FS[0][0]:FS[0][1], :])

        pending_e = []
        row_base = 0
        for ti, (ho0, ho1, la0, la1) in enumerate(THIRDS):
            la_w = la1 - la0
            how = ho1 - ho0
            flat_y = how * OH

            # ---- stage A: einsum, out xwT[po, j, la, m] fp16 ----
            xwT = xwT_pool.tile([128, 3, la_w, M], f16, tag="xwT")
            for g in range(la0, la1, LA_G):
                gn = min(LA_G, la1 - g)
                xg = x_pool.tile([CIN, LA_G, NLON], f16, tag="xg")
                nc.sync.dma_start(out=xg[:, :gn, :], in_=x_in[:, g:g + gn, :])
                for j, (p0, p1) in enumerate(PS):
                    psa = ps_a.tile([128, 512], f32, tag="ps_a")
                    for il in range(gn):
                        nc.tensor.matmul(
                            psa[:p1 - p0, il * M:il * M + M],
                            xg[:, il, p0:p1],
                            w2_sb[:, :],
                            start=True, stop=True)
                    src_ap = _sub_ap(psa[:p1 - p0, 0:1], 0, [[M, gn], [1, M]])
                    dst_ap = xwT[:p1 - p0, j, g - la0:g - la0 + gn, :]
                    if ti == 0:
                        nc.vector.tensor_copy(dst_ap, src_ap)
                    else:
                        nc.scalar.copy(dst_ap, src_ap)

            # ---- stage B: forward DFT, xh[f, t, la*96+m] fp16 ----
            xh = xh_pool.tile([128, 3, la_w * M], f16, tag="xh")
            for t, (f0, f1) in enumerate(FS):
                fsz = f1 - f0
                for l0 in range(0, la_w, BLK_LA):
                    ln = min(BLK_LA, la_w - l0)
                    n = ln * M
                    psb = ps_b.tile([128, 512], f32, tag="ps_b")
                    for j, (p0, p1) in enumerate(PS):
                        nc.tensor.matmul(
                            psb[:fsz, :n],
                            dfwd_sb[:p1 - p0, j, f0:f1],
                            xwT[:p1 - p0, j, l0:l0 + ln, :],
                            start=(j == 0), stop=(j == 2))
                    if ti == 0 and t == 0:
                        nc.vector.tensor_copy(xh[:fsz, t, l0 * M:l0 * M + n],
                                              psb[:fsz, :n])
                    else:
                        nc.scalar.copy(xh[:fsz, t, l0 * M:l0 * M + n],
                                       psb[:fsz, :n])

            # phat DMA after the A/B x-feed so it never delays it on the queue
            # (segment 0's was loaded up-front)
            if ti == 0:
                phat_sb = phat0
                for t, (f0, f1) in enumerate(FS):
                    if t > 0:
                        nc.sync.dma_start(out=phat0[:f1 - f0, t, :, :, :],
                                          in_=phat_in[0, f0:f1, :])
            else:
                phat_sb = phat_pool.tile([128, 3, NPAIR, HOW, 4], f16, tag="phat")
                for t, (f0, f1) in enumerate(FS):
                    nc.sync.dma_start(out=phat_sb[:f1 - f0, t, :, :, :],
                                      in_=phat_in[ti, f0:f1, :])

            # ---- stage E, two segments behind: keeps the PE queue (and the
            # scalar-engine evac FIFO) from stalling ahead of A/B work ----
            if len(pending_e) >= 2:
                emit_e(*pending_e.pop(0))

            # ---- stage D: spectral multiply-accumulate (DVE only) ----
            yh_d = yhd_pool.tile([128, 3, flat_y], f16, tag="yh_d")
            tmp_d = tmpd_pool.tile([128, HOW * OH], f16, tag="tmp_d")
            for t in range(3):
                fsz = FS[t][1] - FS[t][0]
                for ip, (k, dla) in enumerate(NZ):
                    ho_lo = max(ho0, -dla)
                    ho_hi = min(ho1, NLAT - dla)
                    w = ho_hi - ho_lo
                    a = ho_lo + dla - la0
                    hl = ho_lo - ho0
                    assert w > 0 and a >= 0 and a + w <= la_w
                    dims_o = [[OH, w], [4, 12], [1, 4]]
                    in0 = _sub_ap(xh[:fsz, t, 0:1], a * M + k * OH,
                                  [[M, w], [4, 12], [1, 4]])
                    pb = phat_sb[:fsz, t, ip, hl:hl + w, :]
                    in1 = bass.AP(tensor=pb.tensor, offset=pb.offset,
                                  ap=[list(pb.ap[0]), list(pb.ap[1]),
                                      [0, 12], list(pb.ap[2])])
                    if ip == 0:
                        outp = _sub_ap(yh_d[:fsz, t, 0:1], hl * OH, dims_o)
                        nc.vector.tensor_mul(outp, in0, in1)
                    else:
                        tm = _sub_ap(tmp_d[:fsz, 0:1], 0, dims_o)
                        nc.vector.tensor_mul(tm, in0, in1)
                        yflat = _sub_ap(yh_d[:fsz, t, 0:1], hl * OH,
                                        [[1, w * OH]])
                        tflat = _sub_ap(tmp_d[:fsz, 0:1], 0, [[1, w * OH]])
                        nc.vector.tensor_add(yflat, yflat, tflat)

            pending_e.append((yh_d, flat_y, row_base))
            row_base += flat_y
        for pe_args in pending_e:
            emit_e(*pe_args)

    nc.compile()
    return nc


def _get_runner(n_cores=8):
    """Build (once) a jitted shard_map runner for the compiled Bass module."""
    if "runner" in _CACHE:
        return _CACHE["runner"]
    import jax
    import jax.numpy as jnp
    from jax.sharding import Mesh, PartitionSpec, NamedSharding
    from jax.experimental.shard_map import shard_map
    from concourse import bass2jax, mybir

    if "nc" not in _CACHE:
        _CACHE["nc"] = _build_nc()
    nc = _CACHE["nc"]
    bass2jax.install_neuronx_cc_hook()

    partition_name = (nc.partition_id_tensor.name
                      if nc.partition_id_tensor else None)
    in_names, out_names, out_avals = [], [], []
    for alloc in nc.m.functions[0].allocations:
        if not isinstance(alloc, mybir.MemoryLocationSet):
            continue
        name = alloc.memorylocations[0].name
        if alloc.kind == "ExternalInput":
            if name != partition_name:
                in_names.append(name)
        elif alloc.kind == "ExternalOutput":
            out_names.append(name)
            out_avals.append(jax.core.ShapedArray(
                tuple(alloc.tensor_shape), mybir.dt.np(alloc.dtype)))
    n_params = len(in_names)
    n_outs = len(out_avals)
    all_names = in_names + out_names
    if partition_name is not None:
        all_names = all_names + [partition_name]

    def _body(*args):
        operands = list(args)
        if partition_name is not None:
            operands.append(bass2jax.partition_id_tensor())
        outs = bass2jax._bass_exec_p.bind(
            *operands,
            out_avals=tuple(out_avals),
            in_names=tuple(all_names),
            out_names=tuple(out_names),
            lowering_input_output_aliases=(),
            sim_require_finite=True,
            sim_require_nnan=True,
            nc=nc,
        )
        return tuple(outs)

    devices = jax.devices()[:n_cores]
    mesh = Mesh(np.asarray(devices), ("core",))
    spec = PartitionSpec("core")
    sharding = NamedSharding(mesh, spec)
    donate = tuple(range(n_params, n_params + n_outs))
    sharded = jax.jit(
        shard_map(_body, mesh=mesh, in_specs=(spec,) * (n_params + n_outs),
                  out_specs=(spec,) * n_outs, check_rep=False),
        donate_argnums=donate, keep_unused=True)
    zero_shapes = [(n_cores * a.shape[0], *a.shape[1:]) for a in out_avals]
    zero_dtypes = [a.dtype for a in out_avals]
    make_zeros = jax.jit(
        lambda: tuple(jnp.zeros(s, d) for s, d in zip(zero_shapes, zero_dtypes)),
        out_shardings=(sharding,) * n_outs)
    runner = {
        "sharded": sharded, "make_zeros": make_zeros, "sharding": sharding,
        "in_names": in_names, "out_names": out_names, "out_avals": out_avals,
        "n_cores": n_cores,
    }
    _CACHE["runner"] = runner
    return runner


def _device_inputs(x, weight, psi_arrays):
    """Concatenated-global per-parameter arrays, device_put with sharding."""
    import jax
    dfwd, dinv, phat2 = _host_prep(weight, *psi_arrays)
    phat_flat = phat2.reshape(NSEG, FDIM, NPAIR * HOW * 4)
    x16 = x.astype(np.float16)
    per_core = {"x_in": [], "w2_in": [], "dfwd_in": [], "dinv_in": [], "phat_in": []}
    for s in range(8):
        b, ohf = s // 2, s % 2
        o_sl = slice(OH * ohf, OH * ohf + OH)
        # m = k*48 + oh  (k-major)
        w2 = np.ascontiguousarray(
            weight[o_sl].transpose(1, 2, 0).reshape(CIN, M).astype(np.float16))
        per_core["x_in"].append(x16[b])
        per_core["w2_in"].append(w2)
        per_core["dfwd_in"].append(dfwd)
        per_core["dinv_in"].append(dinv)
        per_core["phat_in"].append(phat_flat)
    runner = _get_runner()
    concat = {k: np.concatenate(v, axis=0) for k, v in per_core.items()}
    return [jax.device_put(concat[name], runner["sharding"])
            for name in runner["in_names"]]


def _run_device(dev_in):
    runner = _get_runner()
    zeros = runner["make_zeros"]()
    return runner["sharded"](*dev_in, *zeros)


def kernel(x, weight, bias, psi_vals, k_idx, ho_idx, lat_in_idx, lon_in_idx):
    x = np.ascontiguousarray(np.asarray(x, dtype=np.float32))
    weight = np.asarray(weight, dtype=np.float32)
    bias = np.asarray(bias, dtype=np.float32)
    psi_arrays = (np.asarray(psi_vals), np.asarray(k_idx), np.asarray(ho_idx),
                  np.asarray(lat_in_idx), np.asarray(lon_in_idx))

    dev_in = _device_inputs(x, weight, psi_arrays)
    out_arrs = _run_device(dev_in)
    runner = _get_runner()
    a0 = runner["out_avals"][0]
    res0 = np.asarray(out_arrs[0]).reshape(8, *a0.shape)

    out = np.empty((B, COUT, NLAT, NLON), dtype=np.float32)
    for s in range(8):
        b, ohf = s // 2, s % 2
        r = res0[s]
        parts = []
        base = 0
        for (ho0, ho1, _, _) in THIRDS:
            how = ho1 - ho0
            blk = r[base:base + how * OH].reshape(how, OH, NLON)
            parts.append(blk.transpose(1, 0, 2))
            base += how * OH
        out[b, OH * ohf:OH * ohf + OH] = np.concatenate(parts, axis=1)
    if np.any(bias):
        out += bias[None, :, None, None]
    return out
